# revision 45
# baseline (speedup 1.0000x reference)
"""Batch-parallel attention kernel for 8 TRN2 NeuronCores.

Problem: B=16, S=2048, D=128 full (non-causal) attention, fp32 I/O.
Sharding: batch dim across 8 cores (2 batches/core), no collectives.

Per-core layout trick: everything is computed in "transposed score" space
S^T[k, q] so that no on-device transposes are needed:
  - matmul1: S^T[k,q] = (K^T)[d,k]^T-stationary @ (Q^T)[d,q]-moving,
    contraction over d=128 partitions. Q^T/K^T are prepared on host.
  - softmax numerator exp(scale*S^T) is split 75/25 between two engines
    so the ScalarE exp stream (the old pacer at ~1 elem/cycle/lane)
    drops below the TensorE per-group work:
      * q-columns 0:384 of each 512-wide chunk: ScalarE table exp,
        PSUM->SBUF bf16 (no max subtraction; scores ~N(0,1), max ~7.5).
      * q-columns 384:512: VectorE Schraudolph exp - one tensor_scalar
        computing int16((x*A + B)/2^16): the value-converted int16 IS
        the high half of the classic Schraudolph int32 word, i.e. the
        bf16 bits of exp(x*scale) up to a +-3% PWL ripple, stored
        contiguously; matmul2 loads the tile through a same-size bf16
        bitcast. Zero extra elementwise ops; B carries +0x8000 (+0.5
        post-scale) so the low-half drop rounds to nearest.
    (The Tile framework chains same-tile readers, so the Schraudolph op
    runs right after its group's exp; score PSUM cycles through THREE
    groups so m1 of group g only needs bank g-3 - plenty of slack for
    that serialized read chain.)
  - matmul2: out[q, 0:129] = sum_k expS^T[k,q]^T-stationary @ V_aug[k,:]
    where V_aug = [V | ones]; column 128 accumulates the softmax
    denominator exactly in fp32 PSUM (consistent with the approximated
    numerator, so common-mode exp error cancels). Accumulators are
    packed two per PSUM bank (2 banks total, single-buffered).
  - normalize: on VectorE (ScalarE per-op fixed cost ~500ns is too
    high, GpSimd cannot touch PSUM): per j a reciprocal + a [128,128]
    tensor_scalar multiply. Jobs drain one per k-group (two right after
    a q-chunk's jobs land) so the DVE never sees a lump - a lump would
    delay the Schraudolph -> score-bank-free chain and stall m1 - and
    the acc-bank reads finish before the next q-chunk's first m2
    reuses the bank. The final q-chunk normalizes inline behind its
    last matmuls, with the multiplies split across the idle ScalarE
    and the VectorE so the drain parallelizes.

Steady state: ACT ~930ns/group, DVE ~930ns, TensorE ~920ns
(2x215ns m1 + 8x56ns m2 at ideal issue rates). Startup: input rings
are ordered so first-needed tiles head each DMA ring, and the first
k-group's m1s are split into 256-column halves to start on the first
landed qT half.

PSUM budget: 3 score groups x 2 banks + 2 packed acc banks = 8.
"""

import math
import os

import ml_dtypes
import numpy as np

import concourse.bass as bass
import concourse.mybir as mybir
import concourse.tile as tile
from concourse import bacc
from concourse.bass_utils import run_bass_kernel_spmd

B, S, D = 16, 2048, 128
N_CORES = 8
BPC = B // N_CORES          # batches per core
DA = D + 1                  # V augmented with ones column
QCHUNK = 512                # q processed per inner pipeline chunk
N_QC = S // QCHUNK          # 4
N_KT = S // 128             # 16 k-tiles
KT_GROUPS = [(k, 2) for k in range(0, 16, 2)]
SCALE = 1.0 / float(np.sqrt(D))

Q_EXP = 384                 # q-cols per chunk on ScalarE true exp
Q_SCH = QCHUNK - Q_EXP      # q-cols per chunk on VectorE Schraudolph

# Schraudolph exp: bitcast(int32(x*SCH_A + SCH_B)) ~= exp(x*SCALE).
# A = 2^23 * log2(e) * SCALE; B centers the PWL ripple multiplicatively
# (shift by log2(r*)/2, r* = max (1+f)/2^f) and pre-adds 0x8000 so the
# later bf16 truncation is round-to-nearest.
_L = float(2 ** 23)
_RSTAR = (1.0 / math.log(2.0)) / 2.0 ** (1.0 / math.log(2.0) - 1.0)
SCH_A = _L * math.log2(math.e) * SCALE
SCH_B = 127.0 * _L - math.log2(_RSTAR) / 2.0 * _L + 32768.0
# int16 variant: value-converting (x*A + B)/2^16 yields exactly the high
# 16 bits of the int32 word - i.e. the bf16 bits - STORED CONTIGUOUSLY,
# so matmul2's weight loads need no stride-2 view. B's +0x8000 becomes
# +0.5, giving correct bf16 rounding under truncation or nearest.
SCH_A16 = SCH_A / 65536.0
SCH_B16 = SCH_B / 65536.0

BF16 = mybir.dt.bfloat16
F32 = mybir.dt.float32
I32 = mybir.dt.int32
I16 = mybir.dt.int16

TRACE = bool(os.environ.get("BASS_KERNEL_TRACE"))
LAST_RESULTS = None

_CACHE = {}


def _build():
    nc = bacc.Bacc("TRN2", target_bir_lowering=False, debug=False)

    qT = nc.dram_tensor("qT", [BPC, D, S], BF16, kind="ExternalInput").ap()
    kT = nc.dram_tensor("kT", [BPC, D, S], BF16, kind="ExternalInput").ap()
    vA = nc.dram_tensor("vA", [BPC, S, DA], BF16, kind="ExternalInput").ap()
    out = nc.dram_tensor("out", [BPC, S, D], F32, kind="ExternalOutput").ap()

    with tile.TileContext(nc) as tc:
        with (
            tc.tile_pool(name="qk", bufs=2) as qk_pool,
            tc.tile_pool(name="vp", bufs=2) as v_pool,
            tc.tile_pool(name="warm", bufs=1) as warm_pool,
            tc.tile_pool(name="pexp", bufs=7) as p_pool,
            tc.tile_pool(name="psch", bufs=7) as p32_pool,
            tc.tile_pool(name="outs", bufs=12) as o_pool,
            tc.tile_pool(name="psum_s", bufs=1, space="PSUM") as psum_s,
            tc.tile_pool(name="psum_acc", bufs=1, space="PSUM") as psum_acc,
        ):
            # Pull the ~2.7us exp table load to t=0 so it overlaps the input
            # DMAs instead of stalling the first real exp.
            wtile = warm_pool.tile([128, 1], F32)
            nc.vector.memset(wtile, 0.0)
            nc.scalar.activation(
                wtile, wtile, mybir.ActivationFunctionType.Exp
            )

            QS = S // 4
            batch_tiles = {}

            def load_batch(b):
                kT_sb = [qk_pool.tile([128, QS], BF16, tag=f"kT{h}",
                                      name=f"kT{h}") for h in range(4)]
                qT_sb = [qk_pool.tile([128, QS], BF16, tag=f"qT{h}",
                                      name=f"qT{h}") for h in range(4)]
                v_sb = [v_pool.tile([128, N_KT // 2, DA], BF16, tag=f"v{h}",
                                    name=f"v{h}") for h in range(2)]
                v0 = vA[b][0 : S // 2].rearrange("(t p) d -> p t d", p=128)
                v1 = vA[b][S // 2 : S].rearrange("(t p) d -> p t d", p=128)
                if b == 0:
                    # A ring drains its descriptors in order, so each ring
                    # leads with the tiles the first k-groups need; the
                    # three rings then share HBM bandwidth ~fairly.
                    nc.sync.dma_start(out=kT_sb[0][:, 0:256],
                                      in_=kT[b][:, 0:256])
                    nc.scalar.dma_start(out=qT_sb[0][:, 0:256],
                                        in_=qT[b][:, 0:256])
                    nc.gpsimd.dma_start(out=qT_sb[0][:, 256:512],
                                        in_=qT[b][:, 256:512])
                    nc.sync.dma_start(out=kT_sb[0][:, 256:512],
                                      in_=kT[b][:, 256:512])
                    nc.sync.dma_start(out=kT_sb[1], in_=kT[b][:, QS : 2 * QS])
                    nc.gpsimd.dma_start(out=v_sb[0], in_=v0)
                    nc.sync.dma_start(out=kT_sb[2],
                                      in_=kT[b][:, 2 * QS : 3 * QS])
                    nc.sync.dma_start(out=kT_sb[3], in_=kT[b][:, 3 * QS : S])
                    nc.gpsimd.dma_start(out=qT_sb[1],
                                        in_=qT[b][:, QS : 2 * QS])
                    nc.gpsimd.dma_start(out=v_sb[1], in_=v1)
                    nc.gpsimd.dma_start(out=qT_sb[2],
                                        in_=qT[b][:, 2 * QS : 3 * QS])
                    nc.gpsimd.dma_start(out=qT_sb[3],
                                        in_=qT[b][:, 3 * QS : S])
                else:
                    # Mid-stream prefetch: only the sync + gpsimd rings
                    # (scalar/vector queues are saturated).
                    nc.sync.dma_start(out=kT_sb[0], in_=kT[b][:, 0:QS])
                    nc.gpsimd.dma_start(out=qT_sb[0], in_=qT[b][:, 0:QS])
                    nc.sync.dma_start(out=kT_sb[1], in_=kT[b][:, QS : 2 * QS])
                    nc.gpsimd.dma_start(out=v_sb[0], in_=v0)
                    nc.sync.dma_start(out=kT_sb[2],
                                      in_=kT[b][:, 2 * QS : 3 * QS])
                    nc.gpsimd.dma_start(out=qT_sb[1],
                                        in_=qT[b][:, QS : 2 * QS])
                    nc.sync.dma_start(out=kT_sb[3], in_=kT[b][:, 3 * QS : S])
                    nc.gpsimd.dma_start(out=qT_sb[2],
                                        in_=qT[b][:, 2 * QS : 3 * QS])
                    nc.sync.dma_start(out=qT_sb[3], in_=qT[b][:, 3 * QS : S])
                    nc.gpsimd.dma_start(out=v_sb[1], in_=v1)
                batch_tiles[b] = (kT_sb, qT_sb, v_sb)

            # Deferred normalize, at item granularity so draining never
            # lumps the DVE queue: per j a reciprocal then two [128,64]
            # half-multiplies (the second also issues the out-DMA).
            norm_items = []

            def emit_norm_item(kind, b, qc, acc, j, recip, o_sb, half=0,
                               eng=None, on_act=False):
                a = acc[j]
                if kind == "recip":
                    nc.vector.reciprocal(recip, a[:, D : D + 1])
                    return
                if on_act:
                    # tail only: ScalarE is idle once the exps are done
                    nc.scalar.mul(o_sb, a[:, 0:D], recip[:, 0:1])
                else:
                    nc.vector.tensor_scalar_mul(o_sb, a[:, 0:D], recip)
                r0 = qc * QCHUNK + j * 128
                if eng is None:
                    eng = nc.sync if j % 2 == 0 else nc.gpsimd
                eng.dma_start(out=out[b, r0 : r0 + 128, :], in_=o_sb)

            def queue_norm(b, qc, acc):
                for j in range(4):
                    recip = o_pool.tile([128, 1], F32, tag=f"recip{j % 2}",
                                        name="recip")
                    o_sb = o_pool.tile([128, D], F32, tag=f"o{j % 2}",
                                       name="o_sb")
                    norm_items.append(("recip", b, qc, acc, j, recip, o_sb))
                    norm_items.append(("mul", b, qc, acc, j, recip, o_sb))

            def pop_norm():
                # Front-load: two jobs right after a q-chunk's items land
                # (the DVE absorbs the lump after its Schraudolph op), one
                # per group after - so the acc-bank reads finish before the
                # next q-chunk's first m2 reuses the bank.
                n = 4 if len(norm_items) == 8 else 2
                for _ in range(n):
                    if norm_items:
                        emit_norm_item(*norm_items.pop(0))

            def emit_m2(b, qc, kt0, n_kt, p_tile, p32_t, acc):
                _, _, v_sb = batch_tiles[b]
                # int16 Schraudolph words ARE the bf16 bits, packed
                p32_bf = p32_t[:, :, :].bitcast(BF16)
                final = (b == BPC - 1 and qc == N_QC - 1
                         and kt0 + n_kt == N_KT)
                # ring the final DMAs on sync+scalar: a DMA ringed on
                # gpsimd right before exit adds ~3us of GpSimd DRAIN to
                # the shutdown barrier
                tail_engs = [nc.sync, nc.scalar, nc.sync, nc.scalar]
                for h in range(n_kt):
                    kt = kt0 + h
                    for j in range(4):
                        if j < 3:
                            lhsT = p_tile[:, h, j * 128 : (j + 1) * 128]
                        else:
                            lhsT = p32_bf[:, h, :]
                        # Accumulators are packed two per PSUM bank.
                        # start=True clears has_written for the WHOLE bank,
                        # so only the first slice of each packed bank may
                        # carry it; the second slice's first write lands on
                        # cleared bits and overwrites.
                        nc.tensor.matmul(
                            acc[j],
                            lhsT=lhsT,
                            rhs=v_sb[kt // 8][:, kt % 8, :],
                            start=(kt == 0 and j % 2 == 0),
                            stop=(kt == N_KT - 1),
                        )
                        if final and kt == N_KT - 1:
                            recip = o_pool.tile([128, 1], F32,
                                                tag=f"recip{j % 2}",
                                                name="recip")
                            o_sb = o_pool.tile([128, D], F32,
                                               tag=f"o{j % 2}", name="o_sb")
                            emit_norm_item("recip", b, qc, acc, j, recip,
                                           o_sb)
                            # tail multiplies split across ACT (idle, no
                            # exps left) and DVE so the drain parallelizes
                            emit_norm_item("mul", b, qc, acc, j, recip,
                                           o_sb, eng=tail_engs[j],
                                           on_act=(j % 2 == 0))
                if kt0 + n_kt == N_KT and not final:
                    queue_norm(b, qc, acc)

            # One continuous software pipeline across every (batch, q-chunk,
            # k-group): m2 for group g is emitted after m1 of group g+4, so
            # the in-order PE queue always has independent m1 work while exp
            # runs, and the next q-chunk's acc-bank reuse lands after the
            # previous chunk's normalize items have drained.
            pending = []
            load_batch(0)
            first_group = True
            for b in range(BPC):
                for qc in range(N_QC):
                    kT_sb, qT_sb, _ = batch_tiles[b]
                    acc_t = [
                        psum_acc.tile(
                            [128, 2, DA], F32, tag=f"acc{i}", name=f"acc{i}"
                        )
                        for i in range(2)
                    ]
                    acc = [acc_t[j // 2][:, j % 2, :] for j in range(4)]
                    for kt0, n_kt in KT_GROUPS:
                        ab = "ABC"[(kt0 // 2) % 3]
                        # m2s of group g-4 go FIRST: they are always
                        # ready, so the in-order PE queue keeps streaming
                        # even if this group's m1 weight-load briefly
                        # blocks on the score-bank-free semaphore
                        if len(pending) > 3:
                            emit_m2(*pending.pop(0))
                        s_psum = psum_s.tile(
                            [128, n_kt, QCHUNK], F32, tag=f"s{ab}",
                            name=f"s{ab}",
                        )
                        for h in range(n_kt):
                            kt = kt0 + h
                            lhsT = kT_sb[kt // 4][
                                :, (kt % 4) * 128 : (kt % 4 + 1) * 128
                            ]
                            if first_group:
                                # split into 256-col halves so the first
                                # matmul starts on the first landed qT half
                                for c0 in (0, 256):
                                    nc.tensor.matmul(
                                        s_psum[:, h, c0 : c0 + 256],
                                        lhsT=lhsT,
                                        rhs=qT_sb[qc][:, c0 : c0 + 256],
                                        start=True,
                                        stop=True,
                                    )
                            else:
                                nc.tensor.matmul(
                                    s_psum[:, h, :],
                                    lhsT=lhsT,
                                    rhs=qT_sb[qc],
                                    start=True,
                                    stop=True,
                                )
                        # The Tile framework chains same-tile readers in
                        # emission order, so the second reader pays the
                        # first one's latency. Exp goes FIRST: the chain
                        # lands on the slack-rich DVE stream, and the
                        # 3-bank score rotation absorbs the serialized
                        # m1 -> exp -> sch bank-free chain (~1.4us < 3
                        # group periods).
                        p_tile = p_pool.tile(
                            [128, n_kt, Q_EXP], BF16, tag=f"p{ab}",
                            name=f"p{ab}",
                        )
                        if first_group:
                            # per-kt halves: start exping kt0's scores
                            # while kt1's m1s still run
                            for h in range(n_kt):
                                nc.scalar.activation(
                                    p_tile[:, h : h + 1, :],
                                    s_psum[:, h : h + 1, 0:Q_EXP],
                                    mybir.ActivationFunctionType.Exp,
                                    scale=SCALE,
                                )
                        else:
                            nc.scalar.activation(
                                p_tile,
                                s_psum[:, :, 0:Q_EXP],
                                mybir.ActivationFunctionType.Exp,
                                scale=SCALE,
                            )
                        first_group = False
                        p32_t = p32_pool.tile(
                            [128, n_kt, Q_SCH], I16, tag=f"g{ab}",
                            name=f"g{ab}",
                        )
                        nc.vector.tensor_scalar(
                            p32_t,
                            s_psum[:, :, Q_EXP:QCHUNK],
                            SCH_A16,
                            SCH_B16,
                            op0=mybir.AluOpType.mult,
                            op1=mybir.AluOpType.add,
                        )
                        pending.append((b, qc, kt0, n_kt, p_tile, p32_t, acc))
                        pop_norm(2)
                        # prefetch next batch's inputs once this batch's
                        # first q-chunk is underway
                        if b + 1 < BPC and qc == 1 and kt0 == 6:
                            load_batch(b + 1)
            for args in pending:
                emit_m2(*args)
                pop_norm(2)
            while norm_items:
                pop_norm(2)

    nc.compile()
    return nc


def _get_nc():
    if "nc" not in _CACHE:
        _CACHE["nc"] = _build()
    return _CACHE["nc"]


def kernel(query, key, value):
    global LAST_RESULTS
    bf16 = ml_dtypes.bfloat16
    q = np.ascontiguousarray(
        np.asarray(query, dtype=np.float32).transpose(0, 2, 1)
    ).astype(bf16)
    k = np.ascontiguousarray(
        np.asarray(key, dtype=np.float32).transpose(0, 2, 1)
    ).astype(bf16)
    v = np.asarray(value, dtype=np.float32)
    v_aug = np.concatenate(
        [v, np.ones((B, S, 1), dtype=np.float32)], axis=2
    ).astype(bf16)

    nc = _get_nc()
    in_maps = [
        {
            "qT": q[i * BPC : (i + 1) * BPC],
            "kT": k[i * BPC : (i + 1) * BPC],
            "vA": v_aug[i * BPC : (i + 1) * BPC],
        }
        for i in range(N_CORES)
    ]
    res = run_bass_kernel_spmd(
        nc, in_maps, core_ids=list(range(N_CORES)), trace=TRACE
    )
    LAST_RESULTS = res
    out = np.empty((B, S, D), dtype=np.float32)
    for i in range(N_CORES):
        out[i * BPC : (i + 1) * BPC] = res.results[i]["out"]
    return out


# revision 46
# speedup vs baseline: 1.1834x; 1.1834x over previous
"""Batch-parallel attention kernel for 8 TRN2 NeuronCores.

Problem: B=16, S=2048, D=128 full (non-causal) attention, fp32 I/O.
Sharding: batch dim across 8 cores (2 batches/core), no collectives.

Per-core layout trick: everything is computed in "transposed score" space
S^T[k, q] so that no on-device transposes are needed:
  - matmul1: S^T[k,q] = (K^T)[d,k]^T-stationary @ (Q^T)[d,q]-moving,
    contraction over d=128 partitions. Q^T/K^T are prepared on host.
  - softmax numerator exp(scale*S^T) is split 75/25 between two engines
    so the ScalarE exp stream (the old pacer at ~1 elem/cycle/lane)
    drops below the TensorE per-group work:
      * q-columns 0:384 of each 512-wide chunk: ScalarE table exp,
        PSUM->SBUF bf16 (no max subtraction; scores ~N(0,1), max ~7.5).
      * q-columns 384:512: VectorE Schraudolph exp - one tensor_scalar
        computing int16((x*A + B)/2^16): the value-converted int16 IS
        the high half of the classic Schraudolph int32 word, i.e. the
        bf16 bits of exp(x*scale) up to a +-3% PWL ripple, stored
        contiguously; matmul2 loads the tile through a same-size bf16
        bitcast. Zero extra elementwise ops; B carries +0x8000 (+0.5
        post-scale) so the low-half drop rounds to nearest.
    (The Tile framework chains same-tile readers, so the Schraudolph op
    runs right after its group's exp; score PSUM cycles through THREE
    groups so m1 of group g only needs bank g-3 - plenty of slack for
    that serialized read chain.)
  - matmul2: out[q, 0:129] = sum_k expS^T[k,q]^T-stationary @ V_aug[k,:]
    where V_aug = [V | ones]; column 128 accumulates the softmax
    denominator exactly in fp32 PSUM (consistent with the approximated
    numerator, so common-mode exp error cancels). Accumulators are
    packed two per PSUM bank (2 banks total, single-buffered).
  - normalize: on VectorE (ScalarE per-op fixed cost ~500ns is too
    high, GpSimd cannot touch PSUM): per j a reciprocal + a [128,128]
    tensor_scalar multiply. Jobs drain one per k-group (two right after
    a q-chunk's jobs land) so the DVE never sees a lump - a lump would
    delay the Schraudolph -> score-bank-free chain and stall m1 - and
    the acc-bank reads finish before the next q-chunk's first m2
    reuses the bank. The final q-chunk normalizes inline behind its
    last matmuls, with the multiplies split across the idle ScalarE
    and the VectorE so the drain parallelizes.

Steady state: ACT ~930ns/group, DVE ~930ns, TensorE ~920ns
(2x215ns m1 + 8x56ns m2 at ideal issue rates). Startup: input rings
are ordered so first-needed tiles head each DMA ring, and the first
k-group's m1s are split into 256-column halves to start on the first
landed qT half.

PSUM budget: 3 score groups x 2 banks + 2 packed acc banks = 8.
"""

import math
import os

import ml_dtypes
import numpy as np

import concourse.bass as bass
import concourse.mybir as mybir
import concourse.tile as tile
from concourse import bacc
from concourse.bass_utils import run_bass_kernel_spmd

B, S, D = 16, 2048, 128
N_CORES = 8
BPC = B // N_CORES          # batches per core
DA = D + 1                  # V augmented with ones column
QCHUNK = 512                # q processed per inner pipeline chunk
N_QC = S // QCHUNK          # 4
N_KT = S // 128             # 16 k-tiles
KT_GROUPS = [(k, 2) for k in range(0, 16, 2)]
SCALE = 1.0 / float(np.sqrt(D))

Q_EXP = 384                 # q-cols per chunk on ScalarE true exp
Q_SCH = QCHUNK - Q_EXP      # q-cols per chunk on VectorE Schraudolph

# Schraudolph exp: bitcast(int32(x*SCH_A + SCH_B)) ~= exp(x*SCALE).
# A = 2^23 * log2(e) * SCALE; B centers the PWL ripple multiplicatively
# (shift by log2(r*)/2, r* = max (1+f)/2^f) and pre-adds 0x8000 so the
# later bf16 truncation is round-to-nearest.
_L = float(2 ** 23)
_RSTAR = (1.0 / math.log(2.0)) / 2.0 ** (1.0 / math.log(2.0) - 1.0)
SCH_A = _L * math.log2(math.e) * SCALE
SCH_B = 127.0 * _L - math.log2(_RSTAR) / 2.0 * _L + 32768.0
# int16 variant: value-converting (x*A + B)/2^16 yields exactly the high
# 16 bits of the int32 word - i.e. the bf16 bits - STORED CONTIGUOUSLY,
# so matmul2's weight loads need no stride-2 view. B's +0x8000 becomes
# +0.5, giving correct bf16 rounding under truncation or nearest.
SCH_A16 = SCH_A / 65536.0
SCH_B16 = SCH_B / 65536.0

BF16 = mybir.dt.bfloat16
F32 = mybir.dt.float32
I32 = mybir.dt.int32
I16 = mybir.dt.int16

TRACE = bool(os.environ.get("BASS_KERNEL_TRACE"))
LAST_RESULTS = None

_CACHE = {}


def _build():
    nc = bacc.Bacc("TRN2", target_bir_lowering=False, debug=False)

    qT = nc.dram_tensor("qT", [BPC, D, S], BF16, kind="ExternalInput").ap()
    kT = nc.dram_tensor("kT", [BPC, D, S], BF16, kind="ExternalInput").ap()
    vA = nc.dram_tensor("vA", [BPC, S, DA], BF16, kind="ExternalInput").ap()
    out = nc.dram_tensor("out", [BPC, S, D], F32, kind="ExternalOutput").ap()

    with tile.TileContext(nc) as tc:
        with (
            tc.tile_pool(name="qk", bufs=2) as qk_pool,
            tc.tile_pool(name="vp", bufs=2) as v_pool,
            tc.tile_pool(name="warm", bufs=1) as warm_pool,
            tc.tile_pool(name="pexp", bufs=7) as p_pool,
            tc.tile_pool(name="psch", bufs=7) as p32_pool,
            tc.tile_pool(name="outs", bufs=12) as o_pool,
            tc.tile_pool(name="psum_s", bufs=1, space="PSUM") as psum_s,
            tc.tile_pool(name="psum_acc", bufs=1, space="PSUM") as psum_acc,
        ):
            # Pull the ~2.7us exp table load to t=0 so it overlaps the input
            # DMAs instead of stalling the first real exp.
            wtile = warm_pool.tile([128, 1], F32)
            nc.vector.memset(wtile, 0.0)
            nc.scalar.activation(
                wtile, wtile, mybir.ActivationFunctionType.Exp
            )

            QS = S // 4
            batch_tiles = {}

            def load_batch(b):
                kT_sb = [qk_pool.tile([128, QS], BF16, tag=f"kT{h}",
                                      name=f"kT{h}") for h in range(4)]
                qT_sb = [qk_pool.tile([128, QS], BF16, tag=f"qT{h}",
                                      name=f"qT{h}") for h in range(4)]
                v_sb = [v_pool.tile([128, N_KT // 2, DA], BF16, tag=f"v{h}",
                                    name=f"v{h}") for h in range(2)]
                v0 = vA[b][0 : S // 2].rearrange("(t p) d -> p t d", p=128)
                v1 = vA[b][S // 2 : S].rearrange("(t p) d -> p t d", p=128)
                if b == 0:
                    # A ring drains its descriptors in order, so each ring
                    # leads with the tiles the first k-groups need; the
                    # three rings then share HBM bandwidth ~fairly.
                    nc.sync.dma_start(out=kT_sb[0][:, 0:256],
                                      in_=kT[b][:, 0:256])
                    nc.scalar.dma_start(out=qT_sb[0][:, 0:256],
                                        in_=qT[b][:, 0:256])
                    nc.gpsimd.dma_start(out=qT_sb[0][:, 256:512],
                                        in_=qT[b][:, 256:512])
                    nc.sync.dma_start(out=kT_sb[0][:, 256:512],
                                      in_=kT[b][:, 256:512])
                    nc.sync.dma_start(out=kT_sb[1], in_=kT[b][:, QS : 2 * QS])
                    nc.gpsimd.dma_start(out=v_sb[0], in_=v0)
                    nc.sync.dma_start(out=kT_sb[2],
                                      in_=kT[b][:, 2 * QS : 3 * QS])
                    nc.sync.dma_start(out=kT_sb[3], in_=kT[b][:, 3 * QS : S])
                    nc.gpsimd.dma_start(out=qT_sb[1],
                                        in_=qT[b][:, QS : 2 * QS])
                    nc.gpsimd.dma_start(out=v_sb[1], in_=v1)
                    nc.gpsimd.dma_start(out=qT_sb[2],
                                        in_=qT[b][:, 2 * QS : 3 * QS])
                    nc.gpsimd.dma_start(out=qT_sb[3],
                                        in_=qT[b][:, 3 * QS : S])
                else:
                    # Mid-stream prefetch: only the sync + gpsimd rings
                    # (scalar/vector queues are saturated).
                    nc.sync.dma_start(out=kT_sb[0], in_=kT[b][:, 0:QS])
                    nc.gpsimd.dma_start(out=qT_sb[0], in_=qT[b][:, 0:QS])
                    nc.sync.dma_start(out=kT_sb[1], in_=kT[b][:, QS : 2 * QS])
                    nc.gpsimd.dma_start(out=v_sb[0], in_=v0)
                    nc.sync.dma_start(out=kT_sb[2],
                                      in_=kT[b][:, 2 * QS : 3 * QS])
                    nc.gpsimd.dma_start(out=qT_sb[1],
                                        in_=qT[b][:, QS : 2 * QS])
                    nc.sync.dma_start(out=kT_sb[3], in_=kT[b][:, 3 * QS : S])
                    nc.gpsimd.dma_start(out=qT_sb[2],
                                        in_=qT[b][:, 2 * QS : 3 * QS])
                    nc.sync.dma_start(out=qT_sb[3], in_=qT[b][:, 3 * QS : S])
                    nc.gpsimd.dma_start(out=v_sb[1], in_=v1)
                batch_tiles[b] = (kT_sb, qT_sb, v_sb)

            # Deferred normalize, at item granularity so draining never
            # lumps the DVE queue: per j a reciprocal then two [128,64]
            # half-multiplies (the second also issues the out-DMA).
            norm_items = []

            def emit_norm_item(kind, b, qc, acc, j, recip, o_sb, half=0,
                               eng=None, on_act=False):
                a = acc[j]
                if kind == "recip":
                    nc.vector.reciprocal(recip, a[:, D : D + 1])
                    return
                if on_act:
                    # tail only: ScalarE is idle once the exps are done
                    nc.scalar.mul(o_sb, a[:, 0:D], recip[:, 0:1])
                else:
                    nc.vector.tensor_scalar_mul(o_sb, a[:, 0:D], recip)
                r0 = qc * QCHUNK + j * 128
                if eng is None:
                    eng = nc.sync if j % 2 == 0 else nc.gpsimd
                eng.dma_start(out=out[b, r0 : r0 + 128, :], in_=o_sb)

            def queue_norm(b, qc, acc):
                for j in range(4):
                    recip = o_pool.tile([128, 1], F32, tag=f"recip{j % 2}",
                                        name="recip")
                    o_sb = o_pool.tile([128, D], F32, tag=f"o{j % 2}",
                                       name="o_sb")
                    norm_items.append(("recip", b, qc, acc, j, recip, o_sb))
                    norm_items.append(("mul", b, qc, acc, j, recip, o_sb))

            def pop_norm():
                # Front-load: two jobs right after a q-chunk's items land
                # (the DVE absorbs the lump after its Schraudolph op), one
                # per group after - so the acc-bank reads finish before the
                # next q-chunk's first m2 reuses the bank.
                n = 4 if len(norm_items) == 8 else 2
                for _ in range(n):
                    if norm_items:
                        emit_norm_item(*norm_items.pop(0))

            def emit_m2(b, qc, kt0, n_kt, p_tile, p32_t, acc):
                _, _, v_sb = batch_tiles[b]
                # int16 Schraudolph words ARE the bf16 bits, packed
                p32_bf = p32_t[:, :, :].bitcast(BF16)
                final = (b == BPC - 1 and qc == N_QC - 1
                         and kt0 + n_kt == N_KT)
                # ring the final DMAs on sync+scalar: a DMA ringed on
                # gpsimd right before exit adds ~3us of GpSimd DRAIN to
                # the shutdown barrier
                tail_engs = [nc.sync, nc.scalar, nc.sync, nc.scalar]
                for h in range(n_kt):
                    kt = kt0 + h
                    for j in range(4):
                        if j < 3:
                            lhsT = p_tile[:, h, j * 128 : (j + 1) * 128]
                        else:
                            lhsT = p32_bf[:, h, :]
                        # Accumulators are packed two per PSUM bank.
                        # start=True clears has_written for the WHOLE bank,
                        # so only the first slice of each packed bank may
                        # carry it; the second slice's first write lands on
                        # cleared bits and overwrites.
                        nc.tensor.matmul(
                            acc[j],
                            lhsT=lhsT,
                            rhs=v_sb[kt // 8][:, kt % 8, :],
                            start=(kt == 0 and j % 2 == 0),
                            stop=(kt == N_KT - 1),
                        )
                        if final and kt == N_KT - 1:
                            recip = o_pool.tile([128, 1], F32,
                                                tag=f"recip{j % 2}",
                                                name="recip")
                            o_sb = o_pool.tile([128, D], F32,
                                               tag=f"o{j % 2}", name="o_sb")
                            emit_norm_item("recip", b, qc, acc, j, recip,
                                           o_sb)
                            # tail multiplies split across ACT (idle, no
                            # exps left) and DVE so the drain parallelizes
                            emit_norm_item("mul", b, qc, acc, j, recip,
                                           o_sb, eng=tail_engs[j],
                                           on_act=(j % 2 == 0))
                if kt0 + n_kt == N_KT and not final:
                    queue_norm(b, qc, acc)

            # One continuous software pipeline across every (batch, q-chunk,
            # k-group): m2 for group g is emitted after m1 of group g+4, so
            # the in-order PE queue always has independent m1 work while exp
            # runs, and the next q-chunk's acc-bank reuse lands after the
            # previous chunk's normalize items have drained.
            pending = []
            load_batch(0)
            first_group = True
            for b in range(BPC):
                for qc in range(N_QC):
                    kT_sb, qT_sb, _ = batch_tiles[b]
                    acc_t = [
                        psum_acc.tile(
                            [128, 2, DA], F32, tag=f"acc{i}", name=f"acc{i}"
                        )
                        for i in range(2)
                    ]
                    acc = [acc_t[j // 2][:, j % 2, :] for j in range(4)]
                    for kt0, n_kt in KT_GROUPS:
                        ab = "ABC"[(kt0 // 2) % 3]
                        s_psum = psum_s.tile(
                            [128, n_kt, QCHUNK], F32, tag=f"s{ab}",
                            name=f"s{ab}",
                        )
                        for h in range(n_kt):
                            kt = kt0 + h
                            lhsT = kT_sb[kt // 4][
                                :, (kt % 4) * 128 : (kt % 4 + 1) * 128
                            ]
                            if first_group:
                                # split into 256-col halves so the first
                                # matmul starts on the first landed qT half
                                for c0 in (0, 256):
                                    nc.tensor.matmul(
                                        s_psum[:, h, c0 : c0 + 256],
                                        lhsT=lhsT,
                                        rhs=qT_sb[qc][:, c0 : c0 + 256],
                                        start=True,
                                        stop=True,
                                    )
                            else:
                                nc.tensor.matmul(
                                    s_psum[:, h, :],
                                    lhsT=lhsT,
                                    rhs=qT_sb[qc],
                                    start=True,
                                    stop=True,
                                )
                        # The Tile framework chains same-tile readers in
                        # emission order, so the second reader pays the
                        # first one's latency. Exp goes FIRST: the chain
                        # lands on the slack-rich DVE stream, and the
                        # 3-bank score rotation absorbs the serialized
                        # m1 -> exp -> sch bank-free chain (~1.4us < 3
                        # group periods).
                        p_tile = p_pool.tile(
                            [128, n_kt, Q_EXP], BF16, tag=f"p{ab}",
                            name=f"p{ab}",
                        )
                        if first_group:
                            # per-kt halves: start exping kt0's scores
                            # while kt1's m1s still run
                            for h in range(n_kt):
                                nc.scalar.activation(
                                    p_tile[:, h : h + 1, :],
                                    s_psum[:, h : h + 1, 0:Q_EXP],
                                    mybir.ActivationFunctionType.Exp,
                                    scale=SCALE,
                                )
                        else:
                            nc.scalar.activation(
                                p_tile,
                                s_psum[:, :, 0:Q_EXP],
                                mybir.ActivationFunctionType.Exp,
                                scale=SCALE,
                            )
                        first_group = False
                        p32_t = p32_pool.tile(
                            [128, n_kt, Q_SCH], I16, tag=f"g{ab}",
                            name=f"g{ab}",
                        )
                        nc.vector.tensor_scalar(
                            p32_t,
                            s_psum[:, :, Q_EXP:QCHUNK],
                            SCH_A16,
                            SCH_B16,
                            op0=mybir.AluOpType.mult,
                            op1=mybir.AluOpType.add,
                        )
                        pending.append((b, qc, kt0, n_kt, p_tile, p32_t, acc))
                        if len(pending) > 3:
                            emit_m2(*pending.pop(0))
                        pop_norm(2)
                        # prefetch next batch's inputs once this batch's
                        # first q-chunk is underway
                        if b + 1 < BPC and qc == 1 and kt0 == 6:
                            load_batch(b + 1)
            for args in pending:
                emit_m2(*args)
                pop_norm(2)
            while norm_items:
                pop_norm(2)

    nc.compile()
    return nc


def _get_nc():
    if "nc" not in _CACHE:
        _CACHE["nc"] = _build()
    return _CACHE["nc"]


def kernel(query, key, value):
    global LAST_RESULTS
    bf16 = ml_dtypes.bfloat16
    q = np.ascontiguousarray(
        np.asarray(query, dtype=np.float32).transpose(0, 2, 1)
    ).astype(bf16)
    k = np.ascontiguousarray(
        np.asarray(key, dtype=np.float32).transpose(0, 2, 1)
    ).astype(bf16)
    v = np.asarray(value, dtype=np.float32)
    v_aug = np.concatenate(
        [v, np.ones((B, S, 1), dtype=np.float32)], axis=2
    ).astype(bf16)

    nc = _get_nc()
    in_maps = [
        {
            "qT": q[i * BPC : (i + 1) * BPC],
            "kT": k[i * BPC : (i + 1) * BPC],
            "vA": v_aug[i * BPC : (i + 1) * BPC],
        }
        for i in range(N_CORES)
    ]
    res = run_bass_kernel_spmd(
        nc, in_maps, core_ids=list(range(N_CORES)), trace=TRACE
    )
    LAST_RESULTS = res
    out = np.empty((B, S, D), dtype=np.float32)
    for i in range(N_CORES):
        out[i * BPC : (i + 1) * BPC] = res.results[i]["out"]
    return out
